# revision 13
# baseline (speedup 1.0000x reference)
"""3-layer GAT on 8 Trainium2 NeuronCores (Bass/Tile).

Strategy (dst-node graph partition):
  - Each core owns a contiguous slice of N/8 dst nodes and all edges into
    them. Per layer, nodes are projected data-parallel with an extended
    weight [W | W@al | W@ar]; per-node table rows are packed into 256-BYTE
    elements (the dma_gather sweet spot: ~3 ns/edge vs ~10 ns/edge for
    512B): layers 1-2 store feat as per-(node,head)-scaled int8 plus bf16
    el and bf16 scale; layer 3 stores bf16 feat + el directly. AllGather
    replicates the table; per-edge rows are fetched with dma_gather from
    two half-tables (keeps indices within int16).
  - Attention uses exp without max-subtraction (shift-invariant softmax,
    |e| small); per-128-edge-tile one-hot matmuls accumulate ex-weighted
    feature sums and softmax denominators into per-block PSUM. The int8
    feat is dequantized for free by the ex*scale multiply on Vector.
  - Epilogue (softmax divide, bias, ELU) runs in bulk per ~10-block group,
    and the next layer's projection of each finished group overlaps the
    remaining edge phase; only the AllGather sits between layers.
"""
import os
import sys
from types import SimpleNamespace

import numpy as np
import ml_dtypes

try:
    from concourse import bass, mybir, tile, bacc  # noqa: F401
except ImportError:  # pragma: no cover
    sys.path.insert(0, "/opt/trn_rl_repo")
    from concourse import bass, mybir, tile, bacc  # noqa: F401
from concourse.bass_utils import run_bass_kernel_spmd

bf16 = ml_dtypes.bfloat16
f32 = np.float32

N = int(os.environ.get("KGAT_N", "50000"))
E = int(os.environ.get("KGAT_E", "800000"))
NEG = 0.2
NCORES = 8
NLOC = N // NCORES
BLK = 128
NBLK = (NLOC + BLK - 1) // BLK
NLOC_PAD = NBLK * BLK
TILE = 128
CH = 32                      # tiles per gather chunk
TCOL = 128                   # int16 cols per table row (256 bytes)
RND = 1.5 * 2.0 ** 23        # f32 round-to-nearest-int magic constant

# src-side split: table part 0 = local blocks [0,PSB), part 1 = [PSB,NBLK);
# each part AllGathers into its own tensor (rows fit int16) and part 0's
# collective fires mid-edge-phase, overlapped with remaining gathers.
PSB = (NBLK + 1) // 2
P_ROWS = [PSB * BLK, (NBLK - PSB) * BLK]
P_SPLIT = PSB * BLK
assert NCORES * max(P_ROWS) < 32768

NGRP_PER_PART = 3
GROUPS = [(int(g[0]), int(g[-1]) + 1)
          for rng in (np.arange(PSB), np.arange(PSB, NBLK))
          for g in np.array_split(rng, min(NGRP_PER_PART, len(rng)))
          if len(g)]
GMAX = max(b1 - b0 for b0, b1 in GROUPS)

# layer configs; table row (int16 cols): quant: [feat i8 x128 (cols 0:64) |
# el bf16 xH (64:64+H) | scale bf16 xH (68:68+H)]; else bf16 [feat | el].
LAYERS = [
    dict(kt=2, H=4, D=32, HD=128, quant=True),
    dict(kt=1, H=4, D=32, HD=128, quant=True),
    dict(kt=1, H=1, D=64, HD=64, quant=False),
]
ELC = 64     # bf16 col where el starts (byte 128)
SCC = 68     # bf16 col where scale starts (quant layers)


def _wrap_idx(vals):
    n = len(vals)
    assert n % 16 == 0
    arr = np.asarray(vals, np.int16).reshape(-1, 16).T
    return np.tile(arr, (8, 1))


def _structure(src, dst):
    """Shared tile schedule + per-core index/one-hot arrays."""
    counts = np.zeros((NCORES, NBLK, 2), np.int64)
    per_core = []
    for k in range(NCORES):
        lo = k * NLOC
        m = (dst >= lo) & (dst < lo + NLOC)
        eidx = np.nonzero(m)[0]
        d_loc = dst[eidx] - lo
        half = ((src[eidx] % NLOC) >= P_SPLIT).astype(np.int64)
        blk = d_loc // BLK
        order = np.lexsort((d_loc, blk, half))
        eidx, d_loc, half, blk = (a[order] for a in (eidx, d_loc, half, blk))
        per_core.append((eidx, d_loc, half, blk))
        np.add.at(counts[k], (blk, half), 1)
    T = np.maximum(np.ceil(counts / TILE).astype(np.int64).max(axis=0), 1)

    tile_block, tile_start, tile_stop = [], [], []
    for h in range(2):
        for b in range(NBLK):
            for t in range(T[b, h]):
                tile_block.append(b)
                tile_start.append(t == 0)
                tile_stop.append(t == T[b, h] - 1)
    S = len(tile_block)
    S_A = int(T[:, 0].sum())

    cores = []
    for k in range(NCORES):
        eidx, d_loc, half, blk = per_core[k]
        src_rows = np.zeros(S * TILE, np.int64)
        oh = np.zeros((128, S * TILE), bf16)
        dcode = np.full((128, S), -1.0, bf16)
        pos = 0
        for h in range(2):
            for b in range(NBLK):
                sel = np.nonzero((blk == b) & (half == h))[0]
                ns = len(sel)
                sl = slice(pos, pos + ns)
                s_glob = src[eidx[sel]]
                loc = s_glob % NLOC
                r = ((s_glob // NLOC) * P_ROWS[h] + loc
                     - (P_SPLIT if h else 0))
                src_rows[sl] = r
                slots = pos + np.arange(ns)
                dc = d_loc[sel] - b * BLK
                oh[slots % 128, (slots // 128) * 128 + dc] = 1.0
                dcode[slots % 128, slots // 128] = dc
                pos += T[b, h] * TILE
        assert src_rows.max() < 32768 and src_rows.min() >= 0
        ohT = np.ascontiguousarray(
            oh.reshape(128, S, TILE).transpose(2, 1, 0)).reshape(
                128, S * TILE).astype(np.int8)
        cores.append(dict(idx_src=_wrap_idx(src_rows), dcode=dcode, ohT=ohT))
    meta = dict(T=T, S=S, S_A=S_A, tile_block=tile_block,
                tile_start=tile_start, tile_stop=tile_stop)
    return meta, cores


def _chunks(t0, t1):
    out = []
    t = t0
    while t < t1:
        c = min(CH, t1 - t)
        out.append((t, c))
        t += c
    return out


def _build_program(meta):
    from concourse.masks import make_identity
    dt = mybir.dt
    Alu = mybir.AluOpType
    Act = mybir.ActivationFunctionType
    S, S_A = meta["S"], meta["S_A"]
    tb, tst, tsp = meta["tile_block"], meta["tile_start"], meta["tile_stop"]

    nc = bacc.Bacc("TRN2", target_bir_lowering=False, debug=False,
                   num_devices=NCORES, num_swdge_queues=4)
    xT_in = nc.dram_tensor("xT", [128, 2 * NLOC_PAD], dt.bfloat16,
                           kind="ExternalInput")
    w_in = [nc.dram_tensor(f"W{i+1}", [128, LAYERS[i]["kt"] * (
        LAYERS[i]["HD"] + 2 * LAYERS[i]["H"])], dt.bfloat16,
        kind="ExternalInput") for i in range(3)]
    b_in = [nc.dram_tensor(f"b{i+1}", [128, LAYERS[i]["HD"]], dt.float32,
                           kind="ExternalInput") for i in range(3)]
    isrc_in = nc.dram_tensor("idx_src", [128, S * 8], dt.int16,
                             kind="ExternalInput")
    dcode_in = nc.dram_tensor("dcode", [128, S], dt.bfloat16,
                              kind="ExternalInput")
    iota_in = nc.dram_tensor("iota", [128, 128], dt.bfloat16,
                             kind="ExternalInput")
    ohT_in = nc.dram_tensor("ohT", [128, S * TILE], dt.int8,
                            kind="ExternalInput")
    out_ext = nc.dram_tensor("out", [128, NBLK * 64], dt.float32,
                             kind="ExternalOutput")

    with tile.TileContext(nc) as tc:
        with (
            tc.tile_pool(name="const", bufs=1) as constp,
            tc.tile_pool(name="xblkp", bufs=3) as xblkp,
            tc.tile_pool(name="stage", bufs=1) as stagep,
            tc.tile_pool(name="epi", bufs=2) as epip,
            tc.tile_pool(name="pgath", bufs=4) as pgath,
            tc.tile_pool(name="pwt", bufs=3) as pwt,
            tc.tile_pool(name="poh", bufs=2) as poh,
            tc.tile_pool(name="pohT", bufs=2) as pohT,
            tc.tile_pool(name="pohT8", bufs=3) as pohT8,
            tc.tile_pool(name="pisb", bufs=4) as pisb,
            tc.tile_pool(name="pest", bufs=3) as pest,
            tc.tile_pool(name="psA", bufs=2, space="PSUM") as psA,
            tc.tile_pool(name="psB", bufs=2, space="PSUM") as psB,
            tc.tile_pool(name="dram", bufs=1, space="DRAM") as dram,
        ):
            ident = constp.tile([128, 128], dt.bfloat16, tag="ident")
            make_identity(nc, ident[:])
            w_sb, b_sb = [], []
            for i, cfg in enumerate(LAYERS):
                nw = cfg["HD"] + 2 * cfg["H"]
                w = constp.tile([128, cfg["kt"], nw], dt.bfloat16,
                                tag=f"w{i}")
                nc.sync.dma_start(out=w[:], in_=w_in[i][:].rearrange(
                    "p (k c) -> p k c", k=cfg["kt"]))
                w_sb.append(w)
                bb = constp.tile([128, cfg["HD"]], dt.float32, tag=f"b{i}")
                nc.sync.dma_start(out=bb[:], in_=b_in[i][:])
                b_sb.append(bb)

            tbl_sb = stagep.tile([128, NBLK, TCOL], dt.int16, tag="tblsb")
            nc.vector.memset(tbl_sb[:], 0.0)
            tbl_bf = tbl_sb[:].bitcast(dt.bfloat16)
            tbl_i8 = tbl_sb[:].bitcast(dt.int8)
            er_sb = stagep.tile([128, NBLK, 4], dt.bfloat16, tag="ers")
            accA = stagep.tile([128, NBLK, 132], dt.float32, tag="accA")
            fstage0 = stagep.tile([128, GMAX, 128], dt.float32, tag="fst0")
            elerst0 = stagep.tile([128, GMAX, 8], dt.float32, tag="elerst0")
            fstage1 = stagep.tile([128, GMAX, 128], dt.float32, tag="fst1")
            elerst1 = stagep.tile([128, GMAX, 8], dt.float32, tag="elerst1")
            PBUFS = [(fstage0, elerst0), (fstage1, elerst1)]

            t_loc = [dram.tile([P_ROWS[0], TCOL], dt.int16, tag="tloc0",
                               name="tloc0"),
                     dram.tile([P_ROWS[1], TCOL], dt.int16, tag="tloc1",
                               name="tloc1")]
            t_full = [dram.tile([NCORES * P_ROWS[0], TCOL], dt.int16,
                                tag="tfull0", name="tfull0"),
                      dram.tile([NCORES * P_ROWS[1], TCOL], dt.int16,
                                tag="tfull1", name="tfull1")]
            dcd = constp.tile([128, S], dt.bfloat16, tag="dcd")
            nc.sync.dma_start(out=dcd[:], in_=dcode_in[:])
            iot = constp.tile([128, 128], dt.bfloat16, tag="iot")
            nc.sync.dma_start(out=iot[:], in_=iota_in[:])

            def proj_stage(li, b0, b1, lhs_of, pp_buf):
                """Projection matmuls + staging for blocks [b0,b1)."""
                cfg = LAYERS[li]
                H, HD, kt, quant = cfg["H"], cfg["HD"], cfg["kt"], cfg["quant"]
                fst, elst = pp_buf
                for b in range(b0, b1):
                    pp = psB.tile([128, HD + 2 * H], dt.float32, tag="proj",
                                  name="projpp", space="PSUM")
                    for k in range(kt):
                        nc.tensor.matmul(pp[:], lhsT=lhs_of(b, k),
                                         rhs=w_sb[li][:, k, :],
                                         start=(k == 0), stop=(k == kt - 1))
                    if quant:
                        nc.scalar.activation(fst[:, b - b0, 0:HD],
                                             pp[:, 0:HD], Act.Copy)
                        nc.vector.tensor_copy(out=elst[:, b - b0, 0:2 * H],
                                              in_=pp[:, HD:HD + 2 * H])
                    else:
                        nc.scalar.activation(tbl_bf[:, b, 0:HD + H],
                                             pp[:, 0:HD + H], Act.Copy)
                        nc.vector.tensor_copy(out=er_sb[:, b, 0:H],
                                              in_=pp[:, HD + H:HD + 2 * H])

            def quant_flush(li, b0, b1, pp_buf):
                """Quantize staged blocks into the int8 table (quant layers)."""
                cfg = LAYERS[li]
                H = cfg["H"]
                G = b1 - b0
                fst, elst = pp_buf
                fv = fst[:, 0:G, :].rearrange("p g (h d) -> p g h d", h=H)
                mx = epip.tile([128, GMAX, 4], dt.float32, tag="mx",
                               name="mxt")
                nc.vector.tensor_reduce(out=mx[:, 0:G, :], in_=fv,
                                        axis=mybir.AxisListType.X,
                                        op=Alu.max,
                                        apply_absolute_value=True)
                nc.vector.tensor_scalar_max(out=mx[:, 0:G, :],
                                            in0=mx[:, 0:G, :],
                                            scalar1=1e-20)
                # scale (bf16, stored in table) then rs = 1/scale
                nc.scalar.activation(tbl_bf[:, b0:b1, SCC:SCC + H],
                                     mx[:, 0:G, :], Act.Copy,
                                     scale=1.0 / 127.0)
                rs = epip.tile([128, GMAX, 4], dt.float32, tag="rs",
                               name="rst")
                nc.vector.reciprocal(out=rs[:, 0:G, :],
                                     in_=tbl_bf[:, b0:b1, SCC:SCC + H])
                # qint = round(feat * rs): mult, then +RND -RND trick
                nc.vector.tensor_tensor(
                    out=fv, in0=fv,
                    in1=rs[:, 0:G, :].rearrange("p g (h o) -> p g h o", h=H)
                    .to_broadcast([128, G, H, cfg["D"]]),
                    op=Alu.mult)
                nc.vector.tensor_scalar_add(out=fv, in0=fv, scalar1=RND)
                nc.vector.tensor_scalar_add(out=fv, in0=fv, scalar1=-RND)
                nc.vector.tensor_copy(
                    out=tbl_i8[:, b0:b1, 0:128].rearrange(
                        "p g (h d) -> p g h d", h=H), in_=fv)
                nc.vector.tensor_copy(out=tbl_bf[:, b0:b1, ELC:ELC + H],
                                      in_=elst[:, 0:G, 0:H])
                nc.vector.tensor_copy(out=er_sb[:, b0:b1, 0:H],
                                      in_=elst[:, 0:G, H:2 * H])

            def store_part(part):
                b0, b1 = (0, PSB) if part == 0 else (PSB, NBLK)
                nc.sync.dma_start(
                    out=t_loc[part][:].rearrange("(b p) c -> p b c", p=128),
                    in_=tbl_sb[:, b0:b1, :])
                nc.gpsimd.collective_compute(
                    "AllGather", mybir.AluOpType.bypass,
                    replica_groups=[list(range(NCORES))],
                    ins=[t_loc[part][:].opt()],
                    outs=[t_full[part][:].opt()])

            def epilogue_range(li, b0, b1):
                """Softmax-divide + bias (+ELU+transpose or output DMA)."""
                cfg = LAYERS[li]
                H, HD = cfg["H"], cfg["HD"]
                G = b1 - b0
                last = li == 2
                dr = epip.tile([128, GMAX, 4], dt.float32, tag="dr",
                               name="drt")
                nc.vector.tensor_scalar_add(out=dr[:, 0:G, 0:H],
                                            in0=accA[:, b0:b1, HD:HD + H],
                                            scalar1=1e-9)
                nc.vector.reciprocal(out=dr[:, 0:G, 0:H],
                                     in_=dr[:, 0:G, 0:H])
                qt = epip.tile([128, GMAX, 128], dt.float32, tag="qt",
                               name="qtt")
                nc.vector.tensor_tensor(
                    out=qt[:, 0:G, 0:HD].rearrange("p g (h d) -> p g h d",
                                                   h=H),
                    in0=accA[:, b0:b1, 0:HD].rearrange("p g (h d) -> p g h d",
                                                       h=H),
                    in1=dr[:, 0:G, 0:H].rearrange("p g (h o) -> p g h o",
                                                  h=H)
                    .to_broadcast([128, G, H, cfg["D"]]),
                    op=Alu.mult)
                nc.vector.tensor_tensor(
                    out=qt[:, 0:G, 0:HD], in0=qt[:, 0:G, 0:HD],
                    in1=b_sb[li][:].rearrange("p (o c) -> p o c", o=1)
                    .to_broadcast([128, G, HD]),
                    op=Alu.add)
                if last:
                    nc.sync.dma_start(
                        out=out_ext[:].rearrange("p (b c) -> p b c",
                                                 c=64)[:, b0:b1, :],
                        in_=qt[:, 0:G, 0:64])
                    return None
                # elu: relu(q) + exp(min(q,0)) - 1
                m = epip.tile([128, GMAX, 128], dt.float32, tag="m",
                              name="mt")
                nc.vector.tensor_scalar_min(out=m[:, 0:G, 0:HD],
                                            in0=qt[:, 0:G, 0:HD], scalar1=0.0)
                nc.scalar.activation(m[:, 0:G, 0:HD], m[:, 0:G, 0:HD],
                                     Act.Exp)
                hb = epip.tile([128, GMAX, 128], dt.float32, tag="hb",
                               name="hbt")
                nc.vector.scalar_tensor_tensor(
                    out=hb[:, 0:G, 0:HD], in0=qt[:, 0:G, 0:HD], scalar=0.0,
                    in1=m[:, 0:G, 0:HD], op0=Alu.max, op1=Alu.add)
                hbb = epip.tile([128, GMAX, 128], dt.bfloat16, tag="hbb",
                                name="hbbt")
                nc.vector.tensor_scalar_add(out=hbb[:, 0:G, 0:HD],
                                            in0=hb[:, 0:G, 0:HD],
                                            scalar1=-1.0)
                hgrp = epip.tile([128, GMAX * 128], dt.bfloat16, tag="hgrp",
                                 name="hgrpt")
                for b in range(b0, b1):
                    tp = psB.tile([128, 128], dt.bfloat16, tag="tp",
                                  name="tpt", space="PSUM")
                    nc.tensor.transpose(tp[:], hbb[:, b - b0, :], ident[:])
                    nc.scalar.activation(
                        hgrp[:, (b - b0) * 128:(b - b0 + 1) * 128],
                        tp[:], Act.Copy)
                return hgrp

            def edge_phase(li):
                cfg = LAYERS[li]
                H, D, HD, quant = cfg["H"], cfg["D"], cfg["HD"], cfg["quant"]
                rhsN = HD + H
                last = li == 2
                cur = {"psum": None, "b": None, "half": None}
                chunk_no = [0]
                gi = [0]

                pending = []

                def flush_pending():
                    if not pending:
                        return
                    p = pending.pop()
                    if LAYERS[p[0]]["quant"]:
                        quant_flush(*p[:3], p[3])
                    if p[2] == PSB:
                        store_part(0)
                    elif p[2] == NBLK:
                        store_part(1)

                def group_done(b):
                    if gi[0] < len(GROUPS) and b == GROUPS[gi[0]][1] - 1:
                        b0, b1 = GROUPS[gi[0]]
                        flush_pending()
                        hgrp = epilogue_range(li, b0, b1)
                        if not last:
                            nli = li + 1
                            def lhs_of(bb, k, hgrp=hgrp, b0=b0):
                                return hgrp[:, (bb - b0) * 128:
                                            (bb - b0 + 1) * 128]
                            proj_stage(nli, b0, b1, lhs_of,
                                       PBUFS[gi[0] % 2])
                            pending.append((nli, b0, b1, PBUFS[gi[0] % 2]))
                        gi[0] += 1

                def finish_block():
                    ps, b, half = cur["psum"], cur["b"], cur["half"]
                    if ps is None:
                        return
                    if half == 0:
                        nc.scalar.activation(accA[:, b, 0:rhsN], ps[:],
                                             Act.Copy)
                    else:
                        nc.vector.tensor_tensor(out=accA[:, b, 0:rhsN],
                                                in0=ps[:],
                                                in1=accA[:, b, 0:rhsN],
                                                op=Alu.add)
                        group_done(b)
                    cur["psum"] = None

                for (hf, t0, t1) in ((0, 0, S_A), (1, S_A, S)):
                    tblh = t_full[hf][:]
                    for (c0, cn) in _chunks(t0, t1):
                        ni = cn * TILE
                        isb = pisb.tile([128, CH * 8], dt.int16, tag="isrc")
                        nc.sync.dma_start(
                            out=isb[:, 0:cn * 8],
                            in_=isrc_in[:, c0 * 8:c0 * 8 + cn * 8])
                        ohb = poh.tile([128, CH * TILE], dt.bfloat16,
                                       tag="oh")
                        nc.vector.tensor_tensor(
                            out=ohb[:, 0:cn * TILE].rearrange(
                                "p (c j) -> p c j", j=TILE),
                            in0=dcd[:, c0:c0 + cn].rearrange(
                                "p (c o) -> p c o", o=1).to_broadcast(
                                    [128, cn, TILE]),
                            in1=iot[:].rearrange("p (o j) -> p o j",
                                                 o=1).to_broadcast(
                                                     [128, cn, TILE]),
                            op=Alu.is_equal)
                        ohT8 = pohT8.tile([128, CH * TILE], dt.int8,
                                          tag="ohT8")
                        nc.sync.dma_start(
                            out=ohT8[:, 0:cn * TILE],
                            in_=ohT_in[:, c0 * TILE:(c0 + cn) * TILE])
                        ohTb = pohT.tile([128, CH * TILE], dt.bfloat16,
                                         tag="ohT")
                        nc.scalar.activation(ohTb[:, 0:cn * TILE],
                                             ohT8[:, 0:cn * TILE], Act.Copy)
                        gath = pgath.tile([128, CH, TCOL], dt.int16,
                                          tag="gath")
                        nc.gpsimd.dma_gather(
                            out_ap=gath[:, 0:cn, :], in_ap=tblh,
                            idxs_ap=isb[:, 0:cn * 8], num_idxs=ni,
                            num_idxs_reg=ni, elem_size=TCOL,
                            single_packet=False,
                            queue_num=chunk_no[0] % 4)
                        chunk_no[0] += 1
                        gbf = gath[:, 0:cn, :].bitcast(dt.bfloat16)
                        # er[dst] per slot: per-tile ohT.T @ er_block
                        per = psB.tile([128, CH * 4], dt.float32, tag="er",
                                       name="erps", space="PSUM")
                        for t in range(cn):
                            nc.tensor.matmul(
                                per[:, t * H:(t + 1) * H],
                                lhsT=ohTb[:, t * TILE:(t + 1) * TILE],
                                rhs=er_sb[:, tb[c0 + t], 0:H],
                                start=True, stop=True)
                        est = pest.tile([128, CH, 4], dt.float32, tag="est")
                        nc.vector.tensor_tensor(
                            out=est[:, 0:cn, 0:H],
                            in0=gbf[:, :, ELC:ELC + H],
                            in1=per[:, 0:cn * H].rearrange(
                                "p (c h) -> p c h", h=H),
                            op=Alu.add)
                        nc.vector.scalar_tensor_tensor(
                            out=est[:, 0:cn, 0:H], in0=est[:, 0:cn, 0:H],
                            scalar=NEG, in1=est[:, 0:cn, 0:H],
                            op0=Alu.mult, op1=Alu.max)
                        wt = pwt.tile([128, CH, rhsN], dt.bfloat16, tag="wt")
                        nc.scalar.activation(wt[:, 0:cn, HD:HD + H],
                                             est[:, 0:cn, 0:H], Act.Exp)
                        if quant:
                            # exs = ex * scale (dequant folded in)
                            nc.vector.tensor_tensor(
                                out=est[:, 0:cn, 0:H],
                                in0=wt[:, 0:cn, HD:HD + H],
                                in1=gbf[:, :, SCC:SCC + H],
                                op=Alu.mult)
                            fsrc = gath[:, 0:cn, :].bitcast(dt.int8)[
                                :, :, 0:128].rearrange(
                                    "p c (h d) -> p c h d", h=H)
                            mul_in1 = est[:, 0:cn, 0:H].rearrange(
                                "p c (h o) -> p c h o", h=H).to_broadcast(
                                    [128, cn, H, D])
                        else:
                            fsrc = gbf[:, :, 0:HD].rearrange(
                                "p c (h d) -> p c h d", h=H)
                            mul_in1 = wt[:, 0:cn, HD:HD + H].rearrange(
                                "p c (h o) -> p c h o", h=H).to_broadcast(
                                    [128, cn, H, D])
                        nc.vector.tensor_tensor(
                            out=wt[:, 0:cn, 0:HD].rearrange(
                                "p c (h d) -> p c h d", h=H),
                            in0=fsrc, in1=mul_in1, op=Alu.mult)
                        for t in range(cn):
                            g = c0 + t
                            if tst[g]:
                                finish_block()
                                cur["psum"] = psA.tile(
                                    [128, rhsN], dt.float32, tag="agg",
                                    name="aggp", space="PSUM")
                                cur["b"], cur["half"] = tb[g], hf
                            nc.tensor.matmul(
                                cur["psum"][:],
                                lhsT=ohb[:, t * TILE:(t + 1) * TILE],
                                rhs=wt[:, t, 0:rhsN],
                                start=tst[g], stop=tsp[g])
                    finish_block()
                flush_pending()

            # ---- layer 1 projection from streamed xT blocks ----
            def xlhs(b, k):
                xb = xblkp.tile([128, 2, 128], dt.bfloat16, tag="xb",
                                name=f"xb{b}")
                if k == 0:
                    nc.sync.dma_start(
                        out=xb[:],
                        in_=xT_in[:].rearrange("p (k c) -> p k c",
                                               k=2)[:, :, b * BLK:(b + 1) * BLK])
                    xlhs.cache[b] = xb
                return xlhs.cache[b][:, k, :]
            xlhs.cache = {}

            for i, (b0, b1) in enumerate(GROUPS):
                proj_stage(0, b0, b1, xlhs, PBUFS[i % 2])
                quant_flush(0, b0, b1, PBUFS[i % 2])
                if b1 == PSB:
                    store_part(0)
            store_part(1)
            edge_phase(0)      # overlaps layer-2 proj + collectives
            edge_phase(1)      # overlaps layer-3 proj + collectives
            edge_phase(2)      # writes output per group
    nc.finalize()
    return nc


def kernel(**inputs):
    x = np.asarray(inputs["x"], f32)
    src = np.asarray(inputs["src"]).astype(np.int64)
    dst = np.asarray(inputs["dst"]).astype(np.int64)

    meta, cores = _structure(src, dst)

    def wext(W, al, ar):
        W = np.asarray(W, f32)
        al = np.asarray(al, f32)
        ar = np.asarray(ar, f32)
        Hh, Dd = al.shape
        Wl = np.stack([W[:, h * Dd:(h + 1) * Dd] @ al[h] for h in range(Hh)],
                      1)
        Wr = np.stack([W[:, h * Dd:(h + 1) * Dd] @ ar[h] for h in range(Hh)],
                      1)
        return np.concatenate([W, Wl, Wr], axis=1)

    wx = [wext(inputs["W1"], inputs["al1"], inputs["ar1"]),
          wext(inputs["W2"], inputs["al2"], inputs["ar2"]),
          wext(inputs["W3"], inputs["al3"], inputs["ar3"])]
    w_arrs = []
    for i, cfg in enumerate(LAYERS):
        kt, nw = cfg["kt"], cfg["HD"] + 2 * cfg["H"]
        a = np.zeros((128, kt, nw), bf16)
        for k in range(kt):
            a[:, k, :] = wx[i][k * 128:(k + 1) * 128, :].astype(bf16)
        w_arrs.append(a.reshape(128, kt * nw))
    b_arrs = [np.tile(np.asarray(inputs[f"b{i+1}"], f32).reshape(1, -1),
                      (128, 1)) for i in range(3)]

    nc = _build_program(meta)

    iota_arr = np.tile(np.arange(128, dtype=bf16).reshape(1, 128), (128, 1))
    in_maps = []
    for k in range(NCORES):
        lo = k * NLOC
        xT = np.zeros((128, 2, NLOC_PAD), bf16)
        xs = x[lo:lo + NLOC].astype(bf16)
        for kk in range(2):
            xT[:, kk, 0:NLOC] = xs[:, kk * 128:(kk + 1) * 128].T
        in_maps.append({
            "xT": xT.reshape(128, 2 * NLOC_PAD),
            "W1": w_arrs[0], "W2": w_arrs[1], "W3": w_arrs[2],
            "b1": b_arrs[0], "b2": b_arrs[1], "b3": b_arrs[2],
            "idx_src": cores[k]["idx_src"],
            "dcode": cores[k]["dcode"],
            "iota": iota_arr,
            "ohT": cores[k]["ohT"],
        })

    if os.environ.get("KGAT_SIM"):
        from concourse import bass2jax
        results = bass2jax.run_bass_via_pjrt(nc, in_maps, n_cores=NCORES)
        res = SimpleNamespace(results=results, exec_time_ns=None,
                              instructions_and_trace=None)
    else:
        trace = bool(os.environ.get("KGAT_TRACE"))
        res = run_bass_kernel_spmd(nc, in_maps, core_ids=list(range(NCORES)),
                                   trace=trace)
    global LAST_RESULTS
    LAST_RESULTS = res
    out = np.concatenate(
        [res.results[k]["out"].reshape(128, NBLK, 64)
         .transpose(1, 0, 2).reshape(NLOC_PAD, 64)[:NLOC]
         for k in range(NCORES)], axis=0)
    return out.astype(f32)


LAST_RESULTS = None
